# revision 4
# baseline (speedup 1.0000x reference)
"""Multi-head attention kernel for Trainium2 (Bass/Tile), 8-core data-parallel.

Problem: B=1024 batches of F=128 tokens, D=128 features, H=8 heads, dh=16.
  out = softmax(X Wq (X Wk)^T / sqrt(D)) (X Wv) + X Wr   (per head, concat)

Per-core structure (128 batches, ~1.22us/batch steady state; DVE-paced):
  - Host pre-transposes X to XT [D, B, F] bf16. All matmuls bf16.
  - Q^T PACKED [128, F] (no padding). K zero-padded as TWO variants: KE
    (even heads at rows 32j..32j+16) / KO (odd heads at 32j+16..32j+32).
    Score head h (block j=h//2): lhsT=(KE|KO)[32j:32j+32], rhs=Q[32j:...],
    K=32 row tile at a 32-aligned base. Cuts Q/K PSUM evacuation 4F -> 3F.
  - HW rule (verified: violating it wedges the device): all matmuls that
    ever target a PSUM bank must share one row base (tile_position[0]).
  - Tile-framework dependencies are whole-tile granular, so anything that
    must overlap lives in SEPARATE tiles. PSUM map (8 banks):
      scx/scy pools [F,1024] x2 banks each, one tile per PAIR of batches,
      fully packed: bank j%2 holds head-block j for both batches (head
      (j,o) of batch b at col (j%2)*512+(b%2)*256+o*128). Pool rotation
      (bufs=1, X/Y phase) gives exp(pair)/scores(pair+1) overlap.
      qv_a/qv_b [D,512]: [Qpk|KE|KO|V] per batch parity; evacuated in ONE
      DVE copy [p,512] -> bf16 SBUF (the attn@V rhs V rides along free).
      rps [F,512]: R projections, pair-parity ping-pong; evacuated early
      by a DVE copy (ungated by exp) so the final add is SBUF-only.
      adp [F,512]: attn (2x128) + denominators (2x8), batch parity; only
      post-exp ops touch it so the exp gate cannot leak upstream.
  - exp on ScalarE: ONE contiguous instr per pool per PAIR ([p,1024],
    scale=1/sqrt(D) fused, bf16 out) -> Act ~1048ns/batch.
  - Engine budget per batch (ns): DVE 1121 (evac 658 + Rcopy 196 +
    recip 71 + mul 196) = the pace; Act 1048; PE ~750 (24 matmuls);
    GpSimd (no PSUM port) does the SBUF-only final add ow += R.
  - Scheduling: projections+evacuations are emitted THREE pairs ahead so
    queued evacs clear the serial DVE queue before each slot's exp-gated
    tail (recip/mul); otherwise the cycle tail->evac->projs(qv WAR)->
    scores->exp serializes the whole pipeline (costs ~2x).
  - Softmax max-subtraction skipped (|scores|/sqrt(D) < ~1).
  - Output staged [F, B, D] f32 in half-wave tiles (4-batch out DMAs);
    host transposes back.
"""

import numpy as np
import ml_dtypes

import concourse.bass as bass
import concourse.mybir as mybir
import concourse.tile as tile
from concourse import bacc
from concourse.bass_utils import run_bass_kernel_spmd

BF16 = ml_dtypes.bfloat16

N_CORES = 8
B, F, D = 1024, 128, 128
H, DH = 8, 16
BPC = B // N_CORES   # 128 batches per core
GIO = 8              # batches per IO wave (DMA granularity)
SCALE = 1.0 / float(D) ** 0.5


def build_kernel(nc: bass.Bass):
    f32 = mybir.dt.float32
    bf16 = mybir.dt.bfloat16

    xt = nc.dram_tensor("xt", [D, BPC, F], bf16, kind="ExternalInput")
    # [Wq packed | WKE | WKO], each [D,128]
    wqk = nc.dram_tensor("wqk", [D, 3 * D], bf16, kind="ExternalInput")
    wvr = nc.dram_tensor("wvr", [D, 2 * D], bf16, kind="ExternalInput")
    out = nc.dram_tensor("out", [F, BPC, D], f32, kind="ExternalOutput")

    with tile.TileContext(nc) as tc:
        with (
            tc.tile_pool(name="singles", bufs=1) as singles,
            tc.tile_pool(name="xtp", bufs=3) as xtp,
            tc.tile_pool(name="qkvp", bufs=10) as qkvp,
            tc.tile_pool(name="etp", bufs=6) as etp,
            tc.tile_pool(name="smalls", bufs=4) as smalls,
            tc.tile_pool(name="outp", bufs=5) as outp,
            tc.tile_pool(name="scx", bufs=1, space="PSUM") as scx_pool,
            tc.tile_pool(name="scy", bufs=1, space="PSUM") as scy_pool,
            tc.tile_pool(name="qvps", bufs=1, space="PSUM") as qvps_pool,
            tc.tile_pool(name="rps", bufs=1, space="PSUM") as rps_pool,
            tc.tile_pool(name="adps", bufs=1, space="PSUM") as adps_pool,
        ):
            wqk_sb = singles.tile([D, 3 * D], bf16)
            wvr_sb = singles.tile([D, 2 * D], bf16)
            ones_sb = singles.tile([D, 1], bf16)
            nc.vector.memset(ones_sb, 1.0)
            nc.sync.dma_start(out=wqk_sb, in_=wqk[:, :])
            nc.sync.dma_start(out=wvr_sb, in_=wvr[:, :])

            # persistent PSUM tiles
            qv_a = qvps_pool.tile([D, 512], f32)
            qv_b = qvps_pool.tile([D, 512], f32)
            qvp = [qv_a, qv_b]
            rps = rps_pool.tile([F, 512], f32)
            adp = adps_pool.tile([F, 512], f32)
            adpb = adp.rearrange("p (bk c) -> p bk c", bk=2)  # [p,2,256]

            xtw = [None, None, None]   # input wave tiles (3-rotation)
            xtw_first = None     # 2-batch startup tile
            qkv = {}             # batch -> evacuated [Q|KE|KO|V] sbuf tile
            scxy = {}            # batch -> (X score tile, Y score tile)
            et = {}              # batch -> (et_X [p,512], et_Y) sbuf tiles
            rtmp = {}            # pair -> evacuated R sbuf tile
            ow = [None, None, None, None]   # half-wave output tiles

            def emit_in_dma(w):
                t = xtp.tile([D, GIO * F], bf16, tag="xtw")
                nc.sync.dma_start(out=t, in_=xt[:, w * GIO:(w + 1) * GIO, :])
                xtw[w % 3] = t

            def xtb(b):
                if b < 2:
                    return xtw_first[:, b * F:(b + 1) * F]
                return xtw[(b // GIO) % 3][:, (b % GIO) * F:(b % GIO + 1) * F]

            def emit_projs(b):
                qvt = qvp[b % 2]
                for i in range(3):
                    nc.tensor.matmul(
                        qvt[:, i * D:(i + 1) * D],
                        lhsT=wqk_sb[:, i * D:(i + 1) * D],
                        rhs=xtb(b), start=True, stop=True,
                    )
                nc.tensor.matmul(
                    qvt[:, 3 * D:4 * D],
                    lhsT=xtb(b), rhs=wvr_sb[:, 0:D],
                    start=True, stop=True,
                )
                # R slot: pair-parity ping-pong within the R bank
                ro = ((b // 2) % 2) * 2 * D + (b % 2) * D
                nc.tensor.matmul(
                    rps[:, ro:ro + D],
                    lhsT=xtb(b), rhs=wvr_sb[:, D:2 * D],
                    start=True, stop=True,
                )

            def emit_evac(b):
                t = qkvp.tile([D, 512], bf16)
                nc.vector.tensor_copy(t, qvp[b % 2][:, :])
                qkv[b] = t

            def alloc_sc(p):
                tx = scx_pool.tile([F, 1024], f32, tag="sx")
                ty = scy_pool.tile([F, 1024], f32, tag="sy")
                scxy[p] = (tx, ty)

            def emit_scores(b, grp):
                # pair tile: head (j,o) of batch b at col
                # (j%2)*512 + (b%2)*256 + o*128  (bank j%2, base 32j).
                # grp 0 = X pool (heads 0-3), grp 1 = Y pool (heads 4-7):
                # emitted pool-blocked so the X chain never waits exp_Y.
                sb = qkv[b]
                t = scxy[b // 2][grp]
                for h in range(4 * grp, 4 * grp + 4):
                    j, o = divmod(h, 2)
                    c = (j % 2) * 512 + (b % 2) * 256 + o * F
                    nc.tensor.matmul(
                        t[:, c:c + F],
                        lhsT=sb[:, (1 + o) * D:(2 + o) * D][j * 32:(j + 1) * 32, :],
                        rhs=sb[:, 0:D][j * 32:(j + 1) * 32, :],
                        start=True, stop=True,
                        tile_position=(j * 32, 0),
                    )

            def emit_exp(p):
                ts = []
                for t in scxy[p]:
                    e = etp.tile([F, 1024], bf16, tag="et")
                    nc.scalar.activation(
                        e, t[:, :],
                        mybir.ActivationFunctionType.Exp, scale=SCALE,
                    )
                    ts.append(e)
                et[p] = ts
                scxy.pop(p, None)

            def ethead(b, h):
                j, o = divmod(h, 2)
                t = et[b // 2][0] if j < 2 else et[b // 2][1]
                c = (j % 2) * 512 + (b % 2) * 256 + o * F
                return t[:, c:c + F]

            def emit_denoms(b):
                for h in range(H):
                    nc.tensor.matmul(
                        adp[:, (b % 2) * 256 + 128 + h:(b % 2) * 256 + 129 + h],
                        lhsT=ethead(b, h), rhs=ones_sb, start=True, stop=True,
                    )

            def emit_attnv(b):
                sb = qkv[b]
                for h in range(H):
                    nc.tensor.matmul(
                        adp[:, (b % 2) * 256 + h * DH:
                            (b % 2) * 256 + (h + 1) * DH],
                        lhsT=ethead(b, h),
                        rhs=sb[:, 3 * D + h * DH:3 * D + (h + 1) * DH],
                        start=True, stop=True,
                    )

            def emit_rcopy(p):
                t = smalls.tile([F, 2 * D], bf16, tag="rt")
                nc.vector.tensor_copy(t, rps[:, (p % 2) * 2 * D:(p % 2 + 1) * 2 * D])
                rtmp[p] = t

            def emit_tail(p):
                # pair p = (2p, 2p+1): batch-parity attn/den slots in adp
                rc = smalls.tile([F, 2 * H], f32, tag="rc")
                nc.vector.reciprocal(rc, adpb[:, :, 128:128 + H])
                rc_bc = bass.AP(
                    tensor=rc.tensor, offset=rc.offset,
                    ap=[rc.ap[0], [1, 2 * H], [0, DH]],
                )
                h4, g = divmod(2 * p, GIO // 2)
                dst = ow[h4 % 4][:, g * D:(g + 2) * D]
                nc.vector.tensor_mul(dst, adpb[:, :, 0:128], rc_bc)
                nc.gpsimd.tensor_add(dst, dst, rtmp[p])
                rtmp.pop(p, None)

            def emit_out_dma(h4):
                # half-wave granularity: 4 batches per output DMA
                nc.sync.dma_start(
                    out=out[:, h4 * 4:(h4 + 1) * 4, :], in_=ow[h4 % 4]
                )

            # ---- software-pipelined main loop ----
            # 2-batch startup DMA so the pipeline fills fast, then full waves
            xtw_first = xtp.tile([D, 2 * F], bf16, tag="xtf")
            nc.sync.dma_start(out=xtw_first, in_=xt[:, 0:2, :])
            emit_in_dma(0)
            owt = outp.tile([F, 4 * D], f32)
            ow[0] = owt
            emit_in_dma(1)
            NP = BPC // 2
            for s in range(-3, NP + 1):
                b0, b1 = 2 * s, 2 * s + 1
                if s >= 0 and b0 % GIO == 0 and b0 // GIO + 2 < BPC // GIO:
                    emit_in_dma(b0 // GIO + 2)
                if s >= 1 and (b0 - 2) % 4 == 0:
                    owt = outp.tile([F, 4 * D], f32)
                    ow[((b0 - 2) // 4) % 4] = owt
                # rcopy first: frees the R pair-bank before this slot's R projs
                if -1 <= s < NP - 1:
                    emit_rcopy(s + 1)
                # projections + evacuations THREE pairs ahead so queued evacs
                # clear the serial DVE before this slot's exp-gated tail
                if b0 + 6 < BPC:
                    emit_projs(b0 + 6)
                    emit_evac(b0 + 6)
                if b1 + 6 < BPC:
                    emit_projs(b1 + 6)
                    emit_evac(b1 + 6)
                if 0 <= s < NP:
                    alloc_sc(s)
                    emit_scores(b0, 0)
                    emit_scores(b1, 0)
                    emit_scores(b0, 1)
                    emit_scores(b1, 1)
                if 0 <= s < NP:
                    emit_exp(s)
                if s >= 1:
                    emit_denoms(b0 - 2)
                    emit_attnv(b0 - 2)
                    emit_denoms(b1 - 2)
                    emit_attnv(b1 - 2)
                    emit_tail(s - 1)
                    if (b0 - 2) % 4 == 2:
                        emit_out_dma((b0 - 2) // 4)
                qkv.pop(b0 - 4, None)
                qkv.pop(b1 - 4, None)
                et.pop(s - 2, None)

    return nc


def _prep_wqk(Wq: np.ndarray, Wk: np.ndarray) -> np.ndarray:
    """[Wq packed | KE | KO]: KE/KO zero-pad even/odd heads into 32-blocks."""
    ke = np.zeros((D, D), dtype=np.float32)
    ko = np.zeros((D, D), dtype=np.float32)
    for j in range(4):
        ke[:, 32 * j:32 * j + DH] = Wk[:, DH * 2 * j:DH * (2 * j + 1)]
        ko[:, 32 * j + DH:32 * j + 32] = Wk[:, DH * (2 * j + 1):DH * (2 * j + 2)]
    return np.concatenate([Wq, ke, ko], axis=1)


def prep_in_maps(inputs_dict):
    inputs = np.asarray(inputs_dict["inputs"])
    W_query = np.asarray(inputs_dict["W_query"], dtype=np.float32)
    W_key = np.asarray(inputs_dict["W_key"], dtype=np.float32)
    W_value = np.asarray(inputs_dict["W_value"], dtype=np.float32)
    W_res = np.asarray(inputs_dict["W_res"], dtype=np.float32)

    xt_all = np.ascontiguousarray(inputs.transpose(2, 0, 1)).astype(BF16)
    wqk_np = _prep_wqk(W_query, W_key).astype(BF16)
    wvr_np = np.concatenate([W_value, W_res], axis=1).astype(BF16)

    return [
        {
            "xt": np.ascontiguousarray(xt_all[:, c * BPC:(c + 1) * BPC, :]),
            "wqk": wqk_np,
            "wvr": wvr_np,
        }
        for c in range(N_CORES)
    ]


_COMPILED = {}


def _get_compiled():
    if "nc" not in _COMPILED:
        nc = bacc.Bacc(
            "TRN2", target_bir_lowering=False, debug=False, num_devices=N_CORES
        )
        build_kernel(nc)
        nc.compile()
        _COMPILED["nc"] = nc
    return _COMPILED["nc"]


def kernel(inputs, W_query, W_key, W_value, W_res, **kw):
    in_maps = prep_in_maps({
        "inputs": inputs, "W_query": W_query, "W_key": W_key,
        "W_value": W_value, "W_res": W_res,
    })
    nc = _get_compiled()
    res = run_bass_kernel_spmd(nc, in_maps, core_ids=list(range(N_CORES)))
    parts = [r["out"].transpose(1, 0, 2) for r in res.results]
    return np.concatenate(parts, axis=0)


if __name__ == "__main__":
    rng = np.random.default_rng(0)
    inp = {
        "inputs": rng.standard_normal((B, F, D)).astype(np.float32),
        "W_query": (rng.standard_normal((D, D)) * 0.05).astype(np.float32),
        "W_key": (rng.standard_normal((D, D)) * 0.05).astype(np.float32),
        "W_value": (rng.standard_normal((D, D)) * 0.05).astype(np.float32),
        "W_res": (rng.standard_normal((D, D)) * 0.05).astype(np.float32),
    }
    o = kernel(**inp)

    X, Wq, Wk, Wv, Wr = (inp["inputs"], inp["W_query"], inp["W_key"],
                         inp["W_value"], inp["W_res"])
    def proj(x, w):
        y = np.einsum('bfd,de->bfe', x, w)
        return y.reshape(B, F, H, DH).transpose(0, 2, 1, 3)
    Q, K, V, R = proj(X, Wq), proj(X, Wk), proj(X, Wv), proj(X, Wr)
    s = np.einsum('bhqd,bhkd->bhqk', Q, K) * SCALE
    a = np.exp(s); a = a / a.sum(-1, keepdims=True)
    ref = (np.einsum('bhqk,bhkd->bhqd', a, V) + R)
    ref = ref.transpose(0, 2, 1, 3).reshape(B, F, D)
    rel = np.linalg.norm(o - ref) / np.linalg.norm(ref)
    print("out shape", o.shape, o.dtype, "rel err", rel)


# revision 5
# speedup vs baseline: 1.0179x; 1.0179x over previous
"""Multi-head attention kernel for Trainium2 (Bass/Tile), 8-core data-parallel.

v6: pair-packed score tiles -> one contiguous exp instr per pool per PAIR
(Act 1038ns/batch), evacuation-lean engine assignment, tile-granular deps.

Problem: B=1024 batches of F=128 tokens, D=128 features, H=8 heads, dh=16.
  out = softmax(X Wq (X Wk)^T / sqrt(D)) (X Wv) + X Wr   (per head, concat)

Per-core structure (128 batches):
  - Host pre-transposes X to XT [D, B, F] bf16. All matmuls bf16.
  - Q^T PACKED [128, F] (no padding). K zero-padded as TWO variants: KE
    (even heads at rows 32j..32j+16) / KO (odd heads at 32j+16..32j+32).
    Score head h (block j=h//2): lhsT=(KE|KO)[32j:32j+32], rhs=Q[32j:...],
    K=32 row tile at a 32-aligned base. Cuts Q/K evacuation 4F -> 3F.
  - Tile dependencies are whole-tile granular, so anything that must
    overlap lives in SEPARATE tiles:
      X-pool [F,1024] 2 banks (heads 0-3: block j bank j, 2 heads/bank)
      Y-pool [F,1024] 2 banks (heads 4-7)           -> exp_X(b) overlaps
      scores_Y(b) and scores_X(b+1) (pool gives per-batch tiles).
      qv_a/qv_b [D,512] 1 bank each: [Qpk|KE|KO|V] per batch parity;
      evacuated in ONE DVE copy [p,512] -> bf16 SBUF.
      rps [F,512] 1 bank: R projections, pair ping-pong (2x256). Only
      ungated ops touch it (R-proj write, early R-copy read).
      adp [F,512] 1 bank: attn 2x136 slots (128 attn + 8 denom), batch
      parity. Only post-exp ops touch it (attnV/denoms write, recip/mul
      read) so the exp gate can't leak into the proj/evac stream.
  - exp on ScalarE: TWO instrs per batch (X then Y), each [p,2,256] =
    512 elems, scale fused, bf16 out. Act ~1224ns/batch = the pace.
  - DVE ~1121ns/batch: evac 658 + Rcopy 392/pair + recip 142/pair +
    mul 392/pair (writes ow directly).
  - GpSimd (no PSUM port) does the final SBUF-only add: ow += R.
  - Output staged [F, B, D] f32; host transposes back.
"""

import numpy as np
import ml_dtypes

import concourse.bass as bass
import concourse.mybir as mybir
import concourse.tile as tile
from concourse import bacc
from concourse.bass_utils import run_bass_kernel_spmd

BF16 = ml_dtypes.bfloat16

N_CORES = 8
B, F, D = 1024, 128, 128
H, DH = 8, 16
BPC = B // N_CORES   # 128 batches per core
GIO = 8              # batches per IO wave (DMA granularity)
SCALE = 1.0 / float(D) ** 0.5


def build_kernel(nc: bass.Bass):
    f32 = mybir.dt.float32
    bf16 = mybir.dt.bfloat16

    xt = nc.dram_tensor("xt", [D, BPC, F], bf16, kind="ExternalInput")
    # [Wq packed | WKE | WKO], each [D,128]
    wqk = nc.dram_tensor("wqk", [D, 3 * D], bf16, kind="ExternalInput")
    wvr = nc.dram_tensor("wvr", [D, 2 * D], bf16, kind="ExternalInput")
    out = nc.dram_tensor("out", [F, BPC, D], f32, kind="ExternalOutput")

    with tile.TileContext(nc) as tc:
        with (
            tc.tile_pool(name="singles", bufs=1) as singles,
            tc.tile_pool(name="xtp", bufs=3) as xtp,
            tc.tile_pool(name="qkvp", bufs=7) as qkvp,
            tc.tile_pool(name="etp", bufs=6) as etp,
            tc.tile_pool(name="smalls", bufs=4) as smalls,
            tc.tile_pool(name="outp", bufs=5) as outp,
            tc.tile_pool(name="scx", bufs=1, space="PSUM") as scx_pool,
            tc.tile_pool(name="scy", bufs=1, space="PSUM") as scy_pool,
            tc.tile_pool(name="qvps", bufs=1, space="PSUM") as qvps_pool,
            tc.tile_pool(name="rps", bufs=1, space="PSUM") as rps_pool,
            tc.tile_pool(name="adps", bufs=1, space="PSUM") as adps_pool,
        ):
            wqk_sb = singles.tile([D, 3 * D], bf16)
            wvr_sb = singles.tile([D, 2 * D], bf16)
            ones_sb = singles.tile([D, 1], bf16)
            nc.vector.memset(ones_sb, 1.0)
            nc.sync.dma_start(out=wqk_sb, in_=wqk[:, :])
            nc.sync.dma_start(out=wvr_sb, in_=wvr[:, :])

            # persistent PSUM tiles
            qv = qvps_pool.tile([D, 1024], f32)
            rps = rps_pool.tile([F, 512], f32)
            adp = adps_pool.tile([F, 512], f32)
            adpb = adp.rearrange("p (bk c) -> p bk c", bk=2)  # [p,2,256]

            xtw = [None, None, None]   # input wave tiles (3-rotation)
            xtw_first = None     # 2-batch startup tile
            qkv = {}             # batch -> evacuated [Q|KE|KO|V] sbuf tile
            scxy = {}            # batch -> (X score tile, Y score tile)
            et = {}              # batch -> (et_X [p,512], et_Y) sbuf tiles
            rtmp = {}            # pair -> evacuated R sbuf tile
            ow = [None, None, None, None]   # half-wave output tiles

            def emit_in_dma(w):
                t = xtp.tile([D, GIO * F], bf16, tag="xtw")
                nc.sync.dma_start(out=t, in_=xt[:, w * GIO:(w + 1) * GIO, :])
                xtw[w % 3] = t

            def xtb(b):
                if b < 2:
                    return xtw_first[:, b * F:(b + 1) * F]
                return xtw[(b // GIO) % 3][:, (b % GIO) * F:(b % GIO + 1) * F]

            def emit_projs(b):
                po = (b % 2) * 512
                for i in range(3):
                    nc.tensor.matmul(
                        qv[:, po + i * D:po + (i + 1) * D],
                        lhsT=wqk_sb[:, i * D:(i + 1) * D],
                        rhs=xtb(b), start=True, stop=True,
                    )
                nc.tensor.matmul(
                    qv[:, po + 3 * D:po + 4 * D],
                    lhsT=xtb(b), rhs=wvr_sb[:, 0:D],
                    start=True, stop=True,
                )
                # R slot: pair-parity ping-pong within the R bank
                ro = ((b // 2) % 2) * 2 * D + (b % 2) * D
                nc.tensor.matmul(
                    rps[:, ro:ro + D],
                    lhsT=xtb(b), rhs=wvr_sb[:, D:2 * D],
                    start=True, stop=True,
                )

            def emit_evac(p):
                # ONE [p,1024] copy evacuates BOTH batches of the pair
                t = qkvp.tile([D, 1024], bf16)
                nc.vector.tensor_copy(t, qv[:, :])
                qkv[p] = t

            def qkvb(b):
                return qkv[b // 2][:, (b % 2) * 512:(b % 2) * 512 + 512]

            def alloc_sc(p):
                tx = scx_pool.tile([F, 1024], f32, tag="sx")
                ty = scy_pool.tile([F, 1024], f32, tag="sy")
                scxy[p] = (tx, ty)

            def emit_scores(b, grp):
                # pair tile: head (j,o) of batch b at col
                # (j%2)*512 + (b%2)*256 + o*128  (bank j%2, base 32j).
                # grp 0 = X pool (heads 0-3), grp 1 = Y pool (heads 4-7):
                # emitted pool-blocked so the X chain never waits exp_Y.
                sb = qkvb(b)
                t = scxy[b // 2][grp]
                for h in range(4 * grp, 4 * grp + 4):
                    j, o = divmod(h, 2)
                    c = (j % 2) * 512 + (b % 2) * 256 + o * F
                    nc.tensor.matmul(
                        t[:, c:c + F],
                        lhsT=sb[:, (1 + o) * D:(2 + o) * D][j * 32:(j + 1) * 32, :],
                        rhs=sb[:, 0:D][j * 32:(j + 1) * 32, :],
                        start=True, stop=True,
                        tile_position=(j * 32, 0),
                    )

            def emit_exp(p):
                ts = []
                for t in scxy[p]:
                    e = etp.tile([F, 1024], bf16, tag="et")
                    nc.scalar.activation(
                        e, t[:, :],
                        mybir.ActivationFunctionType.Exp, scale=SCALE,
                    )
                    ts.append(e)
                et[p] = ts
                scxy.pop(p, None)

            def ethead(b, h):
                j, o = divmod(h, 2)
                t = et[b // 2][0] if j < 2 else et[b // 2][1]
                c = (j % 2) * 512 + (b % 2) * 256 + o * F
                return t[:, c:c + F]

            def emit_denoms(b):
                for h in range(H):
                    nc.tensor.matmul(
                        adp[:, (b % 2) * 256 + 128 + h:(b % 2) * 256 + 129 + h],
                        lhsT=ethead(b, h), rhs=ones_sb, start=True, stop=True,
                    )

            def emit_attnv(b):
                sb = qkvb(b)
                for h in range(H):
                    nc.tensor.matmul(
                        adp[:, (b % 2) * 256 + h * DH:
                            (b % 2) * 256 + (h + 1) * DH],
                        lhsT=ethead(b, h),
                        rhs=sb[:, 3 * D + h * DH:3 * D + (h + 1) * DH],
                        start=True, stop=True,
                    )

            def emit_rcopy(p):
                t = smalls.tile([F, 2 * D], bf16, tag="rt")
                nc.vector.tensor_copy(t, rps[:, (p % 2) * 2 * D:(p % 2 + 1) * 2 * D])
                rtmp[p] = t

            def emit_tail(p):
                # pair p = (2p, 2p+1): batch-parity attn/den slots in adp
                rc = smalls.tile([F, 2 * H], f32, tag="rc")
                nc.vector.reciprocal(rc, adpb[:, :, 128:128 + H])
                rc_bc = bass.AP(
                    tensor=rc.tensor, offset=rc.offset,
                    ap=[rc.ap[0], [1, 2 * H], [0, DH]],
                )
                h4, g = divmod(2 * p, GIO // 2)
                dst = ow[h4 % 4][:, g * D:(g + 2) * D]
                nc.vector.tensor_mul(dst, adpb[:, :, 0:128], rc_bc)
                nc.gpsimd.tensor_add(dst, dst, rtmp[p])
                rtmp.pop(p, None)

            def emit_out_dma(h4):
                # half-wave granularity: 4 batches per output DMA
                nc.sync.dma_start(
                    out=out[:, h4 * 4:(h4 + 1) * 4, :], in_=ow[h4 % 4]
                )

            # ---- software-pipelined main loop ----
            # 2-batch startup DMA so the pipeline fills fast, then full waves
            xtw_first = xtp.tile([D, 2 * F], bf16, tag="xtf")
            nc.sync.dma_start(out=xtw_first, in_=xt[:, 0:2, :])
            emit_in_dma(0)
            owt = outp.tile([F, 4 * D], f32)
            ow[0] = owt
            emit_in_dma(1)
            NP = BPC // 2
            for s in range(-3, NP + 1):
                b0, b1 = 2 * s, 2 * s + 1
                if s >= 0 and b0 % GIO == 0 and b0 // GIO + 2 < BPC // GIO:
                    emit_in_dma(b0 // GIO + 2)
                if s >= 1 and (b0 - 2) % 4 == 0:
                    owt = outp.tile([F, 4 * D], f32)
                    ow[((b0 - 2) // 4) % 4] = owt
                # rcopy first in the slot: reads the R pair-bank before the
                # same-parity R projections (emitted below) recycle it
                if -1 <= s < NP - 1:
                    emit_rcopy(s + 1)
                # scores first on the in-order PE (their exp-WAR gate opens
                # earlier than the projs' qv-WAR gate)
                if 0 <= s < NP:
                    alloc_sc(s)
                    emit_scores(b0, 0)
                    emit_scores(b1, 0)
                # projections + evacuations FOUR pairs ahead so queued evacs
                # clear the serial DVE before this slot's exp-gated tail
                if b0 + 6 < BPC:
                    emit_projs(b0 + 6)
                if b1 + 6 < BPC:
                    emit_projs(b1 + 6)
                if b1 + 6 < BPC:
                    emit_evac(s + 3)
                if 0 <= s < NP:
                    emit_scores(b0, 1)
                    emit_scores(b1, 1)
                if 0 <= s < NP:
                    emit_exp(s)
                if s >= 1:
                    emit_denoms(b0 - 2)
                    emit_attnv(b0 - 2)
                    emit_denoms(b1 - 2)
                    emit_attnv(b1 - 2)
                    emit_tail(s - 1)
                    if (b0 - 2) % 4 == 2:
                        emit_out_dma((b0 - 2) // 4)
                qkv.pop(s - 2, None)
                et.pop(s - 2, None)

    return nc


def _prep_wqk(Wq: np.ndarray, Wk: np.ndarray) -> np.ndarray:
    """[Wq packed | KE | KO]: KE/KO zero-pad even/odd heads into 32-blocks."""
    ke = np.zeros((D, D), dtype=np.float32)
    ko = np.zeros((D, D), dtype=np.float32)
    for j in range(4):
        ke[:, 32 * j:32 * j + DH] = Wk[:, DH * 2 * j:DH * (2 * j + 1)]
        ko[:, 32 * j + DH:32 * j + 32] = Wk[:, DH * (2 * j + 1):DH * (2 * j + 2)]
    return np.concatenate([Wq, ke, ko], axis=1)


def prep_in_maps(inputs_dict):
    inputs = np.asarray(inputs_dict["inputs"])
    W_query = np.asarray(inputs_dict["W_query"], dtype=np.float32)
    W_key = np.asarray(inputs_dict["W_key"], dtype=np.float32)
    W_value = np.asarray(inputs_dict["W_value"], dtype=np.float32)
    W_res = np.asarray(inputs_dict["W_res"], dtype=np.float32)

    xt_all = np.ascontiguousarray(inputs.transpose(2, 0, 1)).astype(BF16)
    wqk_np = _prep_wqk(W_query, W_key).astype(BF16)
    wvr_np = np.concatenate([W_value, W_res], axis=1).astype(BF16)

    return [
        {
            "xt": np.ascontiguousarray(xt_all[:, c * BPC:(c + 1) * BPC, :]),
            "wqk": wqk_np,
            "wvr": wvr_np,
        }
        for c in range(N_CORES)
    ]


_COMPILED = {}


def _get_compiled():
    if "nc" not in _COMPILED:
        nc = bacc.Bacc(
            "TRN2", target_bir_lowering=False, debug=False, num_devices=N_CORES
        )
        build_kernel(nc)
        nc.compile()
        _COMPILED["nc"] = nc
    return _COMPILED["nc"]


def kernel(inputs, W_query, W_key, W_value, W_res, **kw):
    in_maps = prep_in_maps({
        "inputs": inputs, "W_query": W_query, "W_key": W_key,
        "W_value": W_value, "W_res": W_res,
    })
    nc = _get_compiled()
    res = run_bass_kernel_spmd(nc, in_maps, core_ids=list(range(N_CORES)))
    parts = [r["out"].transpose(1, 0, 2) for r in res.results]
    return np.concatenate(parts, axis=0)


if __name__ == "__main__":
    rng = np.random.default_rng(0)
    inp = {
        "inputs": rng.standard_normal((B, F, D)).astype(np.float32),
        "W_query": (rng.standard_normal((D, D)) * 0.05).astype(np.float32),
        "W_key": (rng.standard_normal((D, D)) * 0.05).astype(np.float32),
        "W_value": (rng.standard_normal((D, D)) * 0.05).astype(np.float32),
        "W_res": (rng.standard_normal((D, D)) * 0.05).astype(np.float32),
    }
    o = kernel(**inp)

    X, Wq, Wk, Wv, Wr = (inp["inputs"], inp["W_query"], inp["W_key"],
                         inp["W_value"], inp["W_res"])
    def proj(x, w):
        y = np.einsum('bfd,de->bfe', x, w)
        return y.reshape(B, F, H, DH).transpose(0, 2, 1, 3)
    Q, K, V, R = proj(X, Wq), proj(X, Wk), proj(X, Wv), proj(X, Wr)
    s = np.einsum('bhqd,bhkd->bhqk', Q, K) * SCALE
    a = np.exp(s); a = a / a.sum(-1, keepdims=True)
    ref = (np.einsum('bhqk,bhkd->bhqd', a, V) + R)
    ref = ref.transpose(0, 2, 1, 3).reshape(B, F, D)
    rel = np.linalg.norm(o - ref) / np.linalg.norm(ref)
    print("out shape", o.shape, o.dtype, "rel err", rel)


# revision 6
# speedup vs baseline: 1.0214x; 1.0035x over previous
"""Multi-head attention kernel for Trainium2 (Bass/Tile), 8-core data-parallel.

v6: pair-packed score tiles -> one contiguous exp instr per pool per PAIR
(Act 1038ns/batch), evacuation-lean engine assignment, tile-granular deps.

Problem: B=1024 batches of F=128 tokens, D=128 features, H=8 heads, dh=16.
  out = softmax(X Wq (X Wk)^T / sqrt(D)) (X Wv) + X Wr   (per head, concat)

Per-core structure (128 batches):
  - Host pre-transposes X to XT [D, B, F] bf16. All matmuls bf16.
  - Q^T PACKED [128, F] (no padding). K zero-padded as TWO variants: KE
    (even heads at rows 32j..32j+16) / KO (odd heads at 32j+16..32j+32).
    Score head h (block j=h//2): lhsT=(KE|KO)[32j:32j+32], rhs=Q[32j:...],
    K=32 row tile at a 32-aligned base. Cuts Q/K evacuation 4F -> 3F.
  - Tile dependencies are whole-tile granular, so anything that must
    overlap lives in SEPARATE tiles:
      X-pool [F,1024] 2 banks (heads 0-3: block j bank j, 2 heads/bank)
      Y-pool [F,1024] 2 banks (heads 4-7)           -> exp_X(b) overlaps
      scores_Y(b) and scores_X(b+1) (pool gives per-batch tiles).
      qv_a/qv_b [D,512] 1 bank each: [Qpk|KE|KO|V] per batch parity;
      evacuated in ONE DVE copy [p,512] -> bf16 SBUF.
      rps [F,512] 1 bank: R projections, pair ping-pong (2x256). Only
      ungated ops touch it (R-proj write, early R-copy read).
      adp [F,512] 1 bank: attn 2x136 slots (128 attn + 8 denom), batch
      parity. Only post-exp ops touch it (attnV/denoms write, recip/mul
      read) so the exp gate can't leak into the proj/evac stream.
  - exp on ScalarE: TWO instrs per batch (X then Y), each [p,2,256] =
    512 elems, scale fused, bf16 out. Act ~1224ns/batch = the pace.
  - DVE ~1121ns/batch: evac 658 + Rcopy 392/pair + recip 142/pair +
    mul 392/pair (writes ow directly).
  - GpSimd (no PSUM port) does the final SBUF-only add: ow += R.
  - Output staged [F, B, D] f32; host transposes back.
"""

import numpy as np
import ml_dtypes

import concourse.bass as bass
import concourse.mybir as mybir
import concourse.tile as tile
from concourse import bacc
from concourse.bass_utils import run_bass_kernel_spmd

BF16 = ml_dtypes.bfloat16

N_CORES = 8
B, F, D = 1024, 128, 128
H, DH = 8, 16
BPC = B // N_CORES   # 128 batches per core
GIO = 8              # batches per IO wave (DMA granularity)
SCALE = 1.0 / float(D) ** 0.5


def build_kernel(nc: bass.Bass):
    f32 = mybir.dt.float32
    bf16 = mybir.dt.bfloat16

    xt = nc.dram_tensor("xt", [D, BPC, F], bf16, kind="ExternalInput")
    # [Wq packed | WKE | WKO], each [D,128]
    wqk = nc.dram_tensor("wqk", [D, 3 * D], bf16, kind="ExternalInput")
    wvr = nc.dram_tensor("wvr", [D, 2 * D], bf16, kind="ExternalInput")
    out = nc.dram_tensor("out", [F, BPC, D], f32, kind="ExternalOutput")

    with tile.TileContext(nc) as tc:
        with (
            tc.tile_pool(name="singles", bufs=1) as singles,
            tc.tile_pool(name="xtp", bufs=3) as xtp,
            tc.tile_pool(name="qkvp", bufs=7) as qkvp,
            tc.tile_pool(name="etp", bufs=6) as etp,
            tc.tile_pool(name="smalls", bufs=4) as smalls,
            tc.tile_pool(name="outp", bufs=5) as outp,
            tc.tile_pool(name="scx", bufs=1, space="PSUM") as scx_pool,
            tc.tile_pool(name="scy", bufs=1, space="PSUM") as scy_pool,
            tc.tile_pool(name="qvps", bufs=1, space="PSUM") as qvps_pool,
            tc.tile_pool(name="rps", bufs=1, space="PSUM") as rps_pool,
            tc.tile_pool(name="adps", bufs=1, space="PSUM") as adps_pool,
        ):
            wqk_sb = singles.tile([D, 3 * D], bf16)
            wvr_sb = singles.tile([D, 2 * D], bf16)
            ones_sb = singles.tile([D, 1], bf16)
            nc.vector.memset(ones_sb, 1.0)
            nc.sync.dma_start(out=wqk_sb, in_=wqk[:, :])
            nc.sync.dma_start(out=wvr_sb, in_=wvr[:, :])

            # persistent PSUM tiles
            qv = qvps_pool.tile([D, 1024], f32)
            rps = rps_pool.tile([F, 512], f32)
            adp = adps_pool.tile([F, 512], f32)
            adpb = adp.rearrange("p (bk c) -> p bk c", bk=2)  # [p,2,256]

            xtw = [None, None, None]   # input wave tiles (3-rotation)
            xtw_first = None     # 2-batch startup tile
            qkv = {}             # batch -> evacuated [Q|KE|KO|V] sbuf tile
            scxy = {}            # batch -> (X score tile, Y score tile)
            et = {}              # batch -> (et_X [p,512], et_Y) sbuf tiles
            rtmp = {}            # pair -> evacuated R sbuf tile
            ow = [None, None, None, None]   # half-wave output tiles

            def emit_in_dma(w):
                t = xtp.tile([D, GIO * F], bf16, tag="xtw")
                nc.sync.dma_start(out=t, in_=xt[:, w * GIO:(w + 1) * GIO, :])
                xtw[w % 3] = t

            def xtb(b):
                if b < 2:
                    return xtw_first[:, b * F:(b + 1) * F]
                return xtw[(b // GIO) % 3][:, (b % GIO) * F:(b % GIO + 1) * F]

            def emit_projs(b):
                po = (b % 2) * 512
                for i in range(3):
                    nc.tensor.matmul(
                        qv[:, po + i * D:po + (i + 1) * D],
                        lhsT=wqk_sb[:, i * D:(i + 1) * D],
                        rhs=xtb(b), start=True, stop=True,
                    )
                nc.tensor.matmul(
                    qv[:, po + 3 * D:po + 4 * D],
                    lhsT=xtb(b), rhs=wvr_sb[:, 0:D],
                    start=True, stop=True,
                )
                # R slot: pair-parity ping-pong within the R bank
                ro = ((b // 2) % 2) * 2 * D + (b % 2) * D
                nc.tensor.matmul(
                    rps[:, ro:ro + D],
                    lhsT=xtb(b), rhs=wvr_sb[:, D:2 * D],
                    start=True, stop=True,
                )

            def emit_evac(p):
                # ONE [p,1024] copy evacuates BOTH batches of the pair
                t = qkvp.tile([D, 1024], bf16)
                nc.vector.tensor_copy(t, qv[:, :])
                qkv[p] = t

            def qkvb(b):
                return qkv[b // 2][:, (b % 2) * 512:(b % 2) * 512 + 512]

            def alloc_sc(p):
                tx = scx_pool.tile([F, 1024], f32, tag="sx")
                ty = scy_pool.tile([F, 1024], f32, tag="sy")
                scxy[p] = (tx, ty)

            def emit_scores(b, grp):
                # pair tile: head (j,o) of batch b at col
                # (j%2)*512 + (b%2)*256 + o*128  (bank j%2, base 32j).
                # grp 0 = X pool (heads 0-3), grp 1 = Y pool (heads 4-7):
                # emitted pool-blocked so the X chain never waits exp_Y.
                sb = qkvb(b)
                t = scxy[b // 2][grp]
                for h in range(4 * grp, 4 * grp + 4):
                    j, o = divmod(h, 2)
                    c = (j % 2) * 512 + (b % 2) * 256 + o * F
                    nc.tensor.matmul(
                        t[:, c:c + F],
                        lhsT=sb[:, (1 + o) * D:(2 + o) * D][j * 32:(j + 1) * 32, :],
                        rhs=sb[:, 0:D][j * 32:(j + 1) * 32, :],
                        start=True, stop=True,
                        tile_position=(j * 32, 0),
                    )

            def emit_exp(p):
                ts = []
                for t in scxy[p]:
                    e = etp.tile([F, 1024], bf16, tag="et")
                    nc.scalar.activation(
                        e, t[:, :],
                        mybir.ActivationFunctionType.Exp, scale=SCALE,
                    )
                    ts.append(e)
                et[p] = ts
                scxy.pop(p, None)

            def ethead(b, h):
                j, o = divmod(h, 2)
                t = et[b // 2][0] if j < 2 else et[b // 2][1]
                c = (j % 2) * 512 + (b % 2) * 256 + o * F
                return t[:, c:c + F]

            def emit_denoms(b):
                for h in range(H):
                    nc.tensor.matmul(
                        adp[:, (b % 2) * 256 + 128 + h:(b % 2) * 256 + 129 + h],
                        lhsT=ethead(b, h), rhs=ones_sb, start=True, stop=True,
                    )

            def emit_attnv(b):
                sb = qkvb(b)
                for h in range(H):
                    nc.tensor.matmul(
                        adp[:, (b % 2) * 256 + h * DH:
                            (b % 2) * 256 + (h + 1) * DH],
                        lhsT=ethead(b, h),
                        rhs=sb[:, 3 * D + h * DH:3 * D + (h + 1) * DH],
                        start=True, stop=True,
                    )

            def emit_rcopy(p):
                t = smalls.tile([F, 2 * D], bf16, tag="rt")
                nc.vector.tensor_copy(t, rps[:, (p % 2) * 2 * D:(p % 2 + 1) * 2 * D])
                rtmp[p] = t

            def emit_tail(p):
                # pair p = (2p, 2p+1): batch-parity attn/den slots in adp
                rc = smalls.tile([F, 2 * H], f32, tag="rc")
                nc.vector.reciprocal(rc, adpb[:, :, 128:128 + H])
                rc_bc = bass.AP(
                    tensor=rc.tensor, offset=rc.offset,
                    ap=[rc.ap[0], [1, 2 * H], [0, DH]],
                )
                if p == BPC // 2 - 1:
                    # drain special case: own small tile, DVE-only tail
                    # (reads R straight from PSUM), 2-batch final DMA
                    owl = outp.tile([F, 2 * D], f32, tag="owl")
                    nc.vector.tensor_mul(owl, adpb[:, :, 0:128], rc_bc)
                    nc.vector.tensor_add(
                        owl, owl, rps[:, (p % 2) * 2 * D:(p % 2 + 1) * 2 * D]
                    )
                    nc.sync.dma_start(out=out[:, 2 * p:2 * p + 2, :], in_=owl)
                    return
                h4, g = divmod(2 * p, GIO // 2)
                dst = ow[h4 % 4][:, g * D:(g + 2) * D]
                nc.vector.tensor_mul(dst, adpb[:, :, 0:128], rc_bc)
                nc.gpsimd.tensor_add(dst, dst, rtmp[p])
                rtmp.pop(p, None)

            def emit_out_dma(h4):
                # half-wave granularity: 4 batches per output DMA
                nc.sync.dma_start(
                    out=out[:, h4 * 4:(h4 + 1) * 4, :], in_=ow[h4 % 4]
                )

            # ---- software-pipelined main loop ----
            # 2-batch startup DMA so the pipeline fills fast, then full waves
            xtw_first = xtp.tile([D, 2 * F], bf16, tag="xtf")
            nc.sync.dma_start(out=xtw_first, in_=xt[:, 0:2, :])
            emit_in_dma(0)
            owt = outp.tile([F, 4 * D], f32)
            ow[0] = owt
            emit_in_dma(1)
            NP = BPC // 2
            for s in range(-3, NP + 1):
                b0, b1 = 2 * s, 2 * s + 1
                if s >= 0 and b0 % GIO == 0 and b0 // GIO + 2 < BPC // GIO:
                    emit_in_dma(b0 // GIO + 2)
                if s >= 1 and (b0 - 2) % 4 == 0:
                    owt = outp.tile([F, 4 * D], f32)
                    ow[((b0 - 2) // 4) % 4] = owt
                # rcopy first in the slot: reads the R pair-bank before the
                # same-parity R projections (emitted below) recycle it
                if -1 <= s < NP - 1:
                    emit_rcopy(s + 1)
                # scores first on the in-order PE (their exp-WAR gate opens
                # earlier than the projs' qv-WAR gate)
                if 0 <= s < NP:
                    alloc_sc(s)
                    emit_scores(b0, 0)
                    emit_scores(b1, 0)
                # projections + evacuations FOUR pairs ahead so queued evacs
                # clear the serial DVE before this slot's exp-gated tail
                if b0 + 6 < BPC:
                    emit_projs(b0 + 6)
                if b1 + 6 < BPC:
                    emit_projs(b1 + 6)
                if b1 + 6 < BPC:
                    emit_evac(s + 3)
                if 0 <= s < NP:
                    emit_scores(b0, 1)
                    emit_scores(b1, 1)
                if 0 <= s < NP:
                    emit_exp(s)
                if s >= 1:
                    emit_denoms(b0 - 2)
                    emit_attnv(b0 - 2)
                    emit_denoms(b1 - 2)
                    emit_attnv(b1 - 2)
                    emit_tail(s - 1)
                    if (b0 - 2) % 4 == 2 and (b0 - 2) // 4 < BPC // 4 - 1:
                        emit_out_dma((b0 - 2) // 4)
                    if s - 1 == NP - 2:
                        # penultimate pair closes the last half-wave tile's
                        # first half; ship batches 124,125 on their own
                        nc.sync.dma_start(
                            out=out[:, BPC - 4:BPC - 2, :],
                            in_=ow[(BPC // 4 - 1) % 4][:, 0:2 * D],
                        )
                qkv.pop(s - 2, None)
                et.pop(s - 2, None)

    return nc


def _prep_wqk(Wq: np.ndarray, Wk: np.ndarray) -> np.ndarray:
    """[Wq packed | KE | KO]: KE/KO zero-pad even/odd heads into 32-blocks."""
    ke = np.zeros((D, D), dtype=np.float32)
    ko = np.zeros((D, D), dtype=np.float32)
    for j in range(4):
        ke[:, 32 * j:32 * j + DH] = Wk[:, DH * 2 * j:DH * (2 * j + 1)]
        ko[:, 32 * j + DH:32 * j + 32] = Wk[:, DH * (2 * j + 1):DH * (2 * j + 2)]
    return np.concatenate([Wq, ke, ko], axis=1)


def prep_in_maps(inputs_dict):
    inputs = np.asarray(inputs_dict["inputs"])
    W_query = np.asarray(inputs_dict["W_query"], dtype=np.float32)
    W_key = np.asarray(inputs_dict["W_key"], dtype=np.float32)
    W_value = np.asarray(inputs_dict["W_value"], dtype=np.float32)
    W_res = np.asarray(inputs_dict["W_res"], dtype=np.float32)

    xt_all = np.ascontiguousarray(inputs.transpose(2, 0, 1)).astype(BF16)
    wqk_np = _prep_wqk(W_query, W_key).astype(BF16)
    wvr_np = np.concatenate([W_value, W_res], axis=1).astype(BF16)

    return [
        {
            "xt": np.ascontiguousarray(xt_all[:, c * BPC:(c + 1) * BPC, :]),
            "wqk": wqk_np,
            "wvr": wvr_np,
        }
        for c in range(N_CORES)
    ]


_COMPILED = {}


def _get_compiled():
    if "nc" not in _COMPILED:
        nc = bacc.Bacc(
            "TRN2", target_bir_lowering=False, debug=False, num_devices=N_CORES
        )
        build_kernel(nc)
        nc.compile()
        _COMPILED["nc"] = nc
    return _COMPILED["nc"]


def kernel(inputs, W_query, W_key, W_value, W_res, **kw):
    in_maps = prep_in_maps({
        "inputs": inputs, "W_query": W_query, "W_key": W_key,
        "W_value": W_value, "W_res": W_res,
    })
    nc = _get_compiled()
    res = run_bass_kernel_spmd(nc, in_maps, core_ids=list(range(N_CORES)))
    parts = [r["out"].transpose(1, 0, 2) for r in res.results]
    return np.concatenate(parts, axis=0)


if __name__ == "__main__":
    rng = np.random.default_rng(0)
    inp = {
        "inputs": rng.standard_normal((B, F, D)).astype(np.float32),
        "W_query": (rng.standard_normal((D, D)) * 0.05).astype(np.float32),
        "W_key": (rng.standard_normal((D, D)) * 0.05).astype(np.float32),
        "W_value": (rng.standard_normal((D, D)) * 0.05).astype(np.float32),
        "W_res": (rng.standard_normal((D, D)) * 0.05).astype(np.float32),
    }
    o = kernel(**inp)

    X, Wq, Wk, Wv, Wr = (inp["inputs"], inp["W_query"], inp["W_key"],
                         inp["W_value"], inp["W_res"])
    def proj(x, w):
        y = np.einsum('bfd,de->bfe', x, w)
        return y.reshape(B, F, H, DH).transpose(0, 2, 1, 3)
    Q, K, V, R = proj(X, Wq), proj(X, Wk), proj(X, Wv), proj(X, Wr)
    s = np.einsum('bhqd,bhkd->bhqk', Q, K) * SCALE
    a = np.exp(s); a = a / a.sum(-1, keepdims=True)
    ref = (np.einsum('bhqk,bhkd->bhqd', a, V) + R)
    ref = ref.transpose(0, 2, 1, 3).reshape(B, F, D)
    rel = np.linalg.norm(o - ref) / np.linalg.norm(ref)
    print("out shape", o.shape, o.dtype, "rel err", rel)


# revision 7
# speedup vs baseline: 1.0254x; 1.0040x over previous
"""Multi-head attention kernel for Trainium2 (Bass/Tile), 8-core data-parallel.

v6: pair-packed score tiles -> one contiguous exp instr per pool per PAIR
(Act 1038ns/batch), evacuation-lean engine assignment, tile-granular deps.

Problem: B=1024 batches of F=128 tokens, D=128 features, H=8 heads, dh=16.
  out = softmax(X Wq (X Wk)^T / sqrt(D)) (X Wv) + X Wr   (per head, concat)

Per-core structure (128 batches):
  - Host pre-transposes X to XT [D, B, F] bf16. All matmuls bf16.
  - Q^T PACKED [128, F] (no padding). K zero-padded as TWO variants: KE
    (even heads at rows 32j..32j+16) / KO (odd heads at 32j+16..32j+32).
    Score head h (block j=h//2): lhsT=(KE|KO)[32j:32j+32], rhs=Q[32j:...],
    K=32 row tile at a 32-aligned base. Cuts Q/K evacuation 4F -> 3F.
  - Tile dependencies are whole-tile granular, so anything that must
    overlap lives in SEPARATE tiles:
      X-pool [F,1024] 2 banks (heads 0-3: block j bank j, 2 heads/bank)
      Y-pool [F,1024] 2 banks (heads 4-7)           -> exp_X(b) overlaps
      scores_Y(b) and scores_X(b+1) (pool gives per-batch tiles).
      qv_a/qv_b [D,512] 1 bank each: [Qpk|KE|KO|V] per batch parity;
      evacuated in ONE DVE copy [p,512] -> bf16 SBUF.
      rps [F,512] 1 bank: R projections, pair ping-pong (2x256). Only
      ungated ops touch it (R-proj write, early R-copy read).
      adp [F,512] 1 bank: attn 2x136 slots (128 attn + 8 denom), batch
      parity. Only post-exp ops touch it (attnV/denoms write, recip/mul
      read) so the exp gate can't leak into the proj/evac stream.
  - exp on ScalarE: TWO instrs per batch (X then Y), each [p,2,256] =
    512 elems, scale fused, bf16 out. Act ~1224ns/batch = the pace.
  - DVE ~1121ns/batch: evac 658 + Rcopy 392/pair + recip 142/pair +
    mul 392/pair (writes ow directly).
  - GpSimd (no PSUM port) does the final SBUF-only add: ow += R.
  - Output staged [F, B, D] f32; host transposes back.
"""

import numpy as np
import ml_dtypes

import concourse.bass as bass
import concourse.mybir as mybir
import concourse.tile as tile
from concourse import bacc
from concourse.bass_utils import run_bass_kernel_spmd

BF16 = ml_dtypes.bfloat16

N_CORES = 8
B, F, D = 1024, 128, 128
H, DH = 8, 16
BPC = B // N_CORES   # 128 batches per core
GIO = 8              # batches per IO wave (DMA granularity)
SCALE = 1.0 / float(D) ** 0.5


def build_kernel(nc: bass.Bass):
    f32 = mybir.dt.float32
    bf16 = mybir.dt.bfloat16

    xt = nc.dram_tensor("xt", [D, BPC, F], bf16, kind="ExternalInput")
    # [Wq packed | WKE | WKO | Wv | Wr], each [D,128] (one weights DMA)
    wqk = nc.dram_tensor("wqk", [D, 5 * D], bf16, kind="ExternalInput")
    out = nc.dram_tensor("out", [F, BPC, D], f32, kind="ExternalOutput")

    with tile.TileContext(nc) as tc:
        with (
            tc.tile_pool(name="singles", bufs=1) as singles,
            tc.tile_pool(name="xtp", bufs=3) as xtp,
            tc.tile_pool(name="qkvp", bufs=7) as qkvp,
            tc.tile_pool(name="etp", bufs=6) as etp,
            tc.tile_pool(name="smalls", bufs=4) as smalls,
            tc.tile_pool(name="outp", bufs=5) as outp,
            tc.tile_pool(name="scx", bufs=1, space="PSUM") as scx_pool,
            tc.tile_pool(name="scy", bufs=1, space="PSUM") as scy_pool,
            tc.tile_pool(name="qvps", bufs=1, space="PSUM") as qvps_pool,
            tc.tile_pool(name="rps", bufs=1, space="PSUM") as rps_pool,
            tc.tile_pool(name="adps", bufs=1, space="PSUM") as adps_pool,
        ):
            w_sb = singles.tile([D, 5 * D], bf16)
            wqk_sb = w_sb[:, 0:3 * D]
            wvr_sb = w_sb[:, 3 * D:5 * D]
            ones_sb = singles.tile([D, 1], bf16)
            nc.vector.memset(ones_sb, 1.0)
            # single weights DMA, issued FIRST in the SP queue (below)

            # persistent PSUM tiles
            qv = qvps_pool.tile([D, 1024], f32)
            rps = rps_pool.tile([F, 512], f32)
            adp = adps_pool.tile([F, 512], f32)
            adpb = adp.rearrange("p (bk c) -> p bk c", bk=2)  # [p,2,256]

            xtw = [None, None, None]   # input wave tiles (3-rotation)
            xtw_first = None     # 2-batch startup tile
            qkv = {}             # batch -> evacuated [Q|KE|KO|V] sbuf tile
            scxy = {}            # batch -> (X score tile, Y score tile)
            et = {}              # batch -> (et_X [p,512], et_Y) sbuf tiles
            rtmp = {}            # pair -> evacuated R sbuf tile
            ow = [None, None, None, None]   # half-wave output tiles

            def emit_in_dma(w):
                t = xtp.tile([D, GIO * F], bf16, tag="xtw")
                nc.sync.dma_start(out=t, in_=xt[:, w * GIO:(w + 1) * GIO, :])
                xtw[w % 3] = t

            def xtb(b):
                if b < 2:
                    return xtw_first[:, b * F:(b + 1) * F]
                return xtw[(b // GIO) % 3][:, (b % GIO) * F:(b % GIO + 1) * F]

            def emit_projs(b):
                po = (b % 2) * 512
                for i in range(3):
                    nc.tensor.matmul(
                        qv[:, po + i * D:po + (i + 1) * D],
                        lhsT=wqk_sb[:, i * D:(i + 1) * D],
                        rhs=xtb(b), start=True, stop=True,
                    )
                nc.tensor.matmul(
                    qv[:, po + 3 * D:po + 4 * D],
                    lhsT=xtb(b), rhs=wvr_sb[:, 0:D],
                    start=True, stop=True,
                )
                # R slot: pair-parity ping-pong within the R bank
                ro = ((b // 2) % 2) * 2 * D + (b % 2) * D
                nc.tensor.matmul(
                    rps[:, ro:ro + D],
                    lhsT=xtb(b), rhs=wvr_sb[:, D:2 * D],
                    start=True, stop=True,
                )

            def emit_evac(p):
                # ONE [p,1024] copy evacuates BOTH batches of the pair
                t = qkvp.tile([D, 1024], bf16)
                nc.vector.tensor_copy(t, qv[:, :])
                qkv[p] = t

            def qkvb(b):
                return qkv[b // 2][:, (b % 2) * 512:(b % 2) * 512 + 512]

            def alloc_sc(p):
                tx = scx_pool.tile([F, 1024], f32, tag="sx")
                ty = scy_pool.tile([F, 1024], f32, tag="sy")
                scxy[p] = (tx, ty)

            def emit_scores(b, grp):
                # pair tile: head (j,o) of batch b at col
                # (j%2)*512 + (b%2)*256 + o*128  (bank j%2, base 32j).
                # grp 0 = X pool (heads 0-3), grp 1 = Y pool (heads 4-7):
                # emitted pool-blocked so the X chain never waits exp_Y.
                sb = qkvb(b)
                t = scxy[b // 2][grp]
                for h in range(4 * grp, 4 * grp + 4):
                    j, o = divmod(h, 2)
                    c = (j % 2) * 512 + (b % 2) * 256 + o * F
                    nc.tensor.matmul(
                        t[:, c:c + F],
                        lhsT=sb[:, (1 + o) * D:(2 + o) * D][j * 32:(j + 1) * 32, :],
                        rhs=sb[:, 0:D][j * 32:(j + 1) * 32, :],
                        start=True, stop=True,
                        tile_position=(j * 32, 0),
                    )

            def emit_exp(p):
                ts = []
                for t in scxy[p]:
                    e = etp.tile([F, 1024], bf16, tag="et")
                    nc.scalar.activation(
                        e, t[:, :],
                        mybir.ActivationFunctionType.Exp, scale=SCALE,
                    )
                    ts.append(e)
                et[p] = ts
                scxy.pop(p, None)

            def ethead(b, h):
                j, o = divmod(h, 2)
                t = et[b // 2][0] if j < 2 else et[b // 2][1]
                c = (j % 2) * 512 + (b % 2) * 256 + o * F
                return t[:, c:c + F]

            def emit_denoms(b):
                for h in range(H):
                    nc.tensor.matmul(
                        adp[:, (b % 2) * 256 + 128 + h:(b % 2) * 256 + 129 + h],
                        lhsT=ethead(b, h), rhs=ones_sb, start=True, stop=True,
                    )

            def emit_attnv(b):
                sb = qkvb(b)
                for h in range(H):
                    nc.tensor.matmul(
                        adp[:, (b % 2) * 256 + h * DH:
                            (b % 2) * 256 + (h + 1) * DH],
                        lhsT=ethead(b, h),
                        rhs=sb[:, 3 * D + h * DH:3 * D + (h + 1) * DH],
                        start=True, stop=True,
                    )

            def emit_rcopy(p):
                t = smalls.tile([F, 2 * D], bf16, tag="rt")
                nc.vector.tensor_copy(t, rps[:, (p % 2) * 2 * D:(p % 2 + 1) * 2 * D])
                rtmp[p] = t

            def emit_tail(p):
                # pair p = (2p, 2p+1): batch-parity attn/den slots in adp
                rc = smalls.tile([F, 2 * H], f32, tag="rc")
                nc.vector.reciprocal(rc, adpb[:, :, 128:128 + H])
                rc_bc = bass.AP(
                    tensor=rc.tensor, offset=rc.offset,
                    ap=[rc.ap[0], [1, 2 * H], [0, DH]],
                )
                if p == BPC // 2 - 1:
                    # drain special case: own small tile, DVE-only tail
                    # (reads R straight from PSUM), 2-batch final DMA
                    owl = outp.tile([F, 2 * D], f32, tag="owl")
                    nc.vector.tensor_mul(owl, adpb[:, :, 0:128], rc_bc)
                    nc.vector.tensor_add(
                        owl, owl, rps[:, (p % 2) * 2 * D:(p % 2 + 1) * 2 * D]
                    )
                    nc.sync.dma_start(out=out[:, 2 * p:2 * p + 2, :], in_=owl)
                    return
                h4, g = divmod(2 * p, GIO // 2)
                dst = ow[h4 % 4][:, g * D:(g + 2) * D]
                nc.vector.tensor_mul(dst, adpb[:, :, 0:128], rc_bc)
                nc.gpsimd.tensor_add(dst, dst, rtmp[p])
                rtmp.pop(p, None)

            def emit_out_dma(h4):
                # half-wave granularity: 4 batches per output DMA
                nc.sync.dma_start(
                    out=out[:, h4 * 4:(h4 + 1) * 4, :], in_=ow[h4 % 4]
                )

            # ---- software-pipelined main loop ----
            # 2-batch startup DMA so the pipeline fills fast, then full waves
            nc.sync.dma_start(out=w_sb, in_=wqk[:, :])
            xtw_first = xtp.tile([D, 2 * F], bf16, tag="xtf")
            nc.sync.dma_start(out=xtw_first, in_=xt[:, 0:2, :])
            emit_in_dma(0)
            owt = outp.tile([F, 4 * D], f32)
            ow[0] = owt
            emit_in_dma(1)
            NP = BPC // 2
            for s in range(-3, NP + 1):
                b0, b1 = 2 * s, 2 * s + 1
                if s >= 0 and b0 % GIO == 0 and b0 // GIO + 2 < BPC // GIO:
                    emit_in_dma(b0 // GIO + 2)
                if s >= 1 and (b0 - 2) % 4 == 0:
                    owt = outp.tile([F, 4 * D], f32)
                    ow[((b0 - 2) // 4) % 4] = owt
                # rcopy first in the slot: reads the R pair-bank before the
                # same-parity R projections (emitted below) recycle it
                if -1 <= s < NP - 1:
                    emit_rcopy(s + 1)
                # scores first on the in-order PE (their exp-WAR gate opens
                # earlier than the projs' qv-WAR gate)
                if 0 <= s < NP:
                    alloc_sc(s)
                    emit_scores(b0, 0)
                    emit_scores(b1, 0)
                # projections + evacuations FOUR pairs ahead so queued evacs
                # clear the serial DVE before this slot's exp-gated tail
                if b0 + 6 < BPC:
                    emit_projs(b0 + 6)
                if b1 + 6 < BPC:
                    emit_projs(b1 + 6)
                if b1 + 6 < BPC:
                    emit_evac(s + 3)
                if 0 <= s < NP:
                    emit_scores(b0, 1)
                    emit_scores(b1, 1)
                if 0 <= s < NP:
                    emit_exp(s)
                if s >= 1:
                    emit_denoms(b0 - 2)
                    emit_attnv(b0 - 2)
                    emit_denoms(b1 - 2)
                    emit_attnv(b1 - 2)
                    emit_tail(s - 1)
                    if (b0 - 2) % 4 == 2 and (b0 - 2) // 4 < BPC // 4 - 1:
                        emit_out_dma((b0 - 2) // 4)
                    if s - 1 == NP - 2:
                        # penultimate pair closes the last half-wave tile's
                        # first half; ship batches 124,125 on their own
                        nc.sync.dma_start(
                            out=out[:, BPC - 4:BPC - 2, :],
                            in_=ow[(BPC // 4 - 1) % 4][:, 0:2 * D],
                        )
                qkv.pop(s - 2, None)
                et.pop(s - 2, None)

    return nc


def _prep_wqk(Wq: np.ndarray, Wk: np.ndarray) -> np.ndarray:
    """[Wq packed | KE | KO]: KE/KO zero-pad even/odd heads into 32-blocks."""
    ke = np.zeros((D, D), dtype=np.float32)
    ko = np.zeros((D, D), dtype=np.float32)
    for j in range(4):
        ke[:, 32 * j:32 * j + DH] = Wk[:, DH * 2 * j:DH * (2 * j + 1)]
        ko[:, 32 * j + DH:32 * j + 32] = Wk[:, DH * (2 * j + 1):DH * (2 * j + 2)]
    return np.concatenate([Wq, ke, ko], axis=1)


def prep_in_maps(inputs_dict):
    inputs = np.asarray(inputs_dict["inputs"])
    W_query = np.asarray(inputs_dict["W_query"], dtype=np.float32)
    W_key = np.asarray(inputs_dict["W_key"], dtype=np.float32)
    W_value = np.asarray(inputs_dict["W_value"], dtype=np.float32)
    W_res = np.asarray(inputs_dict["W_res"], dtype=np.float32)

    xt_all = np.ascontiguousarray(inputs.transpose(2, 0, 1)).astype(BF16)
    wqk_np = np.concatenate(
        [_prep_wqk(W_query, W_key), W_value, W_res], axis=1
    ).astype(BF16)

    return [
        {
            "xt": np.ascontiguousarray(xt_all[:, c * BPC:(c + 1) * BPC, :]),
            "wqk": wqk_np,
        }
        for c in range(N_CORES)
    ]


_COMPILED = {}


def _get_compiled():
    if "nc" not in _COMPILED:
        nc = bacc.Bacc(
            "TRN2", target_bir_lowering=False, debug=False, num_devices=N_CORES
        )
        build_kernel(nc)
        nc.compile()
        _COMPILED["nc"] = nc
    return _COMPILED["nc"]


def kernel(inputs, W_query, W_key, W_value, W_res, **kw):
    in_maps = prep_in_maps({
        "inputs": inputs, "W_query": W_query, "W_key": W_key,
        "W_value": W_value, "W_res": W_res,
    })
    nc = _get_compiled()
    res = run_bass_kernel_spmd(nc, in_maps, core_ids=list(range(N_CORES)))
    parts = [r["out"].transpose(1, 0, 2) for r in res.results]
    return np.concatenate(parts, axis=0)


if __name__ == "__main__":
    rng = np.random.default_rng(0)
    inp = {
        "inputs": rng.standard_normal((B, F, D)).astype(np.float32),
        "W_query": (rng.standard_normal((D, D)) * 0.05).astype(np.float32),
        "W_key": (rng.standard_normal((D, D)) * 0.05).astype(np.float32),
        "W_value": (rng.standard_normal((D, D)) * 0.05).astype(np.float32),
        "W_res": (rng.standard_normal((D, D)) * 0.05).astype(np.float32),
    }
    o = kernel(**inp)

    X, Wq, Wk, Wv, Wr = (inp["inputs"], inp["W_query"], inp["W_key"],
                         inp["W_value"], inp["W_res"])
    def proj(x, w):
        y = np.einsum('bfd,de->bfe', x, w)
        return y.reshape(B, F, H, DH).transpose(0, 2, 1, 3)
    Q, K, V, R = proj(X, Wq), proj(X, Wk), proj(X, Wv), proj(X, Wr)
    s = np.einsum('bhqd,bhkd->bhqk', Q, K) * SCALE
    a = np.exp(s); a = a / a.sum(-1, keepdims=True)
    ref = (np.einsum('bhqk,bhkd->bhqd', a, V) + R)
    ref = ref.transpose(0, 2, 1, 3).reshape(B, F, D)
    rel = np.linalg.norm(o - ref) / np.linalg.norm(ref)
    print("out shape", o.shape, o.dtype, "rel err", rel)
